# revision 29
# baseline (speedup 1.0000x reference)
"""CNF step (3-layer tanh MLP vector field + exact divergence) on 8 trn2 cores.

Math: for each sample x in R^64 (x's last column is the logp channel, replaced
by scalar t in the MLP input):
    h1 = tanh([x, t] @ W1 + b1);  h2 = tanh(h1 @ W2 + b2)
    dx = (h2 @ W3 + b3) / 2
    div = trace(J) where J = d(dx)/dx
Closed form for the jacobian trace (avoids jacrev entirely):
    div = (1/2) * d1^T K d2,  d1 = 1-h1^2, d2 = 1-h2^2,
    K[m,j] = W2[m,j] * sum_i W1[i,m] W3[j,i]
K is a pure function of the (launch-invariant) weights, folded on host once.
All O(batch) compute runs on device.

Device layout is fully transposed (features on partitions, batch on free dim):
weights serve directly as matmul lhsT operands, so the kernel needs zero
on-device transposes. Host pre-transposes x (layout prep) and re-transposes
the output. Matmuls run in float32r (single-pass, full-rate) -- every producer
of a matmul operand writes f32r explicitly, as the BIR verifier requires.

Sharding: pure data parallel, batch 2048 -> 8 cores x 256 samples.
"""

import numpy as np

import bass_rust
import concourse.bass as bass
import concourse.tile as tile
from concourse import mybir
from concourse.bass_utils import run_bass_kernel_spmd

# This walrus build only encodes a single sem-wait per instruction; Tile's
# scheduler freely emits instructions carrying 2-3 waits and codegen dies
# with "Too many sync wait commands". Hoist extra waits onto single-wait
# EventSemaphore carrier instructions placed immediately before the
# multi-wait instruction on the same engine (semantically identical:
# engines execute in order, all waits still precede the op).
_orig_add_instruction = tile.TileContext._add_instruction


def _split_waits(tc_self, inst):
    si = getattr(inst, "sync_info", None)
    if (
        si is not None
        and si.on_wait
        and len(si.on_wait) > 1
        and inst.engine != mybir.EngineType.Unassigned
    ):
        waits = list(si.on_wait)
        upds = list(si.on_update) if si.on_update else []
        for w in waits[:-1]:
            carrier = mybir.InstEventSemaphore(
                name=tc_self.nc.get_next_instruction_name(),
                engine=inst.engine,
                ins=[],
                outs=[],
                sync_info=mybir.SyncInfo(on_wait=[w], on_update=[]),
                bass_nofuse=True,
            )
            _orig_add_instruction(tc_self, carrier)
        inst.sync_info = mybir.SyncInfo(on_wait=[waits[-1]], on_update=upds)


def _patched_add_instruction(self, inst):
    _split_waits(self, inst)
    _orig_add_instruction(self, inst)


tile.TileContext._add_instruction = _patched_add_instruction


# Minimal kernel tail. Tile's stock tail (drain + all-engine barrier + sem
# clear + barrier) exists to reset semaphore/DMA state for the next
# execution -- but the Bass preamble at the START of every execution already
# clears the whole kernel sem range (range(150,256)) and resets DMA state,
# so the tail only needs to hold the NEFF open until every outstanding sem
# (including the output-store DMA completions) reaches its terminal value.
# Emit that as a chain of single-wait drains on SP (the walrus build's
# 1-wait-per-instruction limit again).
def _patched_drain_and_barrier(self, tick_clock, wait_clock):
    nc = self.nc
    drain_inst = nc.sync.drain()
    wait_clock.add_sem_waits(
        drain_inst.ins, bass_rust.ScopedClock({None: tick_clock.global_clock})
    )
    si = drain_inst.ins.sync_info
    waits = list(si.on_wait) if si is not None and si.on_wait else []
    if len(waits) > 1:
        upds = list(si.on_update) if si.on_update else []
        drain_inst.ins.sync_info = mybir.SyncInfo(on_wait=[waits[0]], on_update=upds)
        for w in waits[1:]:
            extra = nc.sync.drain()
            extra.ins.sync_info = mybir.SyncInfo(on_wait=[w], on_update=[])
    popped = nc._tile_sem_poison_stack.pop()
    assert popped is self._sem_poison


tile.TileContext._drain_and_barrier = _patched_drain_and_barrier


# The stock compile pipeline passes --enable-ldw-opt=false, which forces every
# matmul to pay a serial 128-cycle LDWEIGHTS. Enable the double-buffered
# weight-load path.
import concourse.bass_utils as _bu

_orig_bvo = _bu.bir_verify_and_optimise


def _bvo_ldwopt(*a, **k):
    import subprocess as _sp

    orig_run = _bu.run_command

    def run_command_ldw(cmd, **kw):
        cmd = [c.replace("--enable-ldw-opt=false", "--enable-ldw-opt=true") for c in cmd]
        return orig_run(cmd, **kw)

    _bu.run_command = run_command_ldw
    try:
        return _orig_bvo(*a, **k)
    finally:
        _bu.run_command = orig_run


_bu.bir_verify_and_optimise = _orig_bvo  # placeholder, replaced below
_bu.bir_verify_and_optimise = _bvo_ldwopt

F32 = mybir.dt.float32
F32R = mybir.dt.float32r
AF = mybir.ActivationFunctionType
OP = mybir.AluOpType

B, D, H = 2048, 64, 512
NCORES = 8
BS = B // NCORES  # 256 samples per core
NCH = H // 128    # 4 feature chunks of 128


def _build_program():
    nc = bass.Bass(monotonic_sem_count=0)

    # packed block 1 (critical path, 65 rows -- no zero-padding rows):
    # cols 0:256 xaT, 256:768 w1
    # packed block 2: cols 0:256 w3 chunks, 256:264 b1|b2, 264:266 b3|ones
    PK1W, PK2W = 768, 266
    pk1 = nc.declare_dram_parameter("pk1", [D + 1, PK1W], F32, isOutput=False)
    pk2 = nc.declare_dram_parameter("pk2", [128, PK2W], F32, isOutput=False)
    w2 = nc.declare_dram_parameter("w2", [H, H], F32, isOutput=False)
    km = nc.declare_dram_parameter("km", [H, H], F32, isOutput=False)
    out_dx = nc.declare_dram_parameter("out_dx", [D, BS], F32, isOutput=True)
    out_dv = nc.declare_dram_parameter("out_dv", [1, BS], F32, isOutput=True)

    with tile.TileContext(nc) as tc:
        with (
            tc.tile_pool(name="wts", bufs=1) as wts,
            tc.tile_pool(name="acts", bufs=1) as acts,
            tc.tile_pool(name="ps_z", bufs=4, space="PSUM") as ps_z,
            tc.tile_pool(name="ps_a", bufs=2, space="PSUM") as ps_a,
            tc.tile_pool(name="ps_o", bufs=1, space="PSUM") as ps_o,
        ):
            # ---- loads ---------------------------------------------------
            # one packed transfer for everything small (1 issue on SP), the
            # 1MB w2 as 4 column-block transfers issued from the otherwise
            # idle gpsimd engine (consumers need w2 one column block at a
            # time, so L2 starts at 1/4-arrival)
            w2_sb = wts.tile([128, NCH, H], F32, tag="w2_sb")
            nc.sync.dma_start(
                out=w2_sb[:, :, 0:128].bitcast(F32R),
                in_=w2[:, 0:128].rearrange("(k p) j -> p k j", p=128).bitcast(F32R),
            )
            pk1_sb = wts.tile([D + 1, PK1W], F32, tag="pk1_sb")
            nc.sync.dma_start(out=pk1_sb.bitcast(F32R), in_=pk1[:, :].bitcast(F32R))
            xaT_sb = pk1_sb[:, 0:BS]
            w1_sb = pk1_sb[:, BS : BS + H]

            pk2_sb = wts.tile([128, PK2W], F32, tag="pk2_sb")
            nc.gpsimd.dma_start(out=pk2_sb.bitcast(F32R), in_=pk2[:, :].bitcast(F32R))
            w3_sb = pk2_sb[:, 0 : NCH * D].rearrange("p (k d) -> p k d", d=D)
            b12_sb = pk2_sb[:, NCH * D : NCH * D + 2 * NCH]
            b3o_sb = pk2_sb[:, PK2W - 2 : PK2W]

            # single sync queue = explicit transfer priority. Interleave so
            # each consumer's block lands just-in-time: L2 chunk c needs
            # w2 block c, the divergence matmuls need km blocks later.
            km_sb = wts.tile([128, NCH, H], F32, tag="km_sb")

            def _w2cb(c):
                nc.sync.dma_start(
                    out=w2_sb[:, :, 128 * c : 128 * (c + 1)].bitcast(F32R),
                    in_=w2[:, 128 * c : 128 * (c + 1)]
                    .rearrange("(k p) j -> p k j", p=128)
                    .bitcast(F32R),
                )

            def _kmcb(c):
                nc.sync.dma_start(
                    out=km_sb[:, :, 128 * c : 128 * (c + 1)].bitcast(F32R),
                    in_=km[:, 128 * c : 128 * (c + 1)]
                    .rearrange("(k p) j -> p k j", p=128)
                    .bitcast(F32R),
                )

            _w2cb(1)
            _w2cb(2)
            _w2cb(3)
            _kmcb(0)
            _kmcb(1)
            _kmcb(2)
            _kmcb(3)

            # warm the ACT table (tanh set) before any data dependency
            warm = acts.tile([1, 1], F32, tag="warm")
            nc.vector.memset(warm, 0.0)
            nc.scalar.activation(warm, warm, AF.Tanh)

            # ---- layer 1: h1^T = tanh(W1^T @ xaT + b1) ------------------
            h1_sb = acts.tile([128, NCH * BS], F32, tag="h1_sb")
            for c in range(NCH):
                z1 = ps_z.tile([128, BS], F32, tag="z", name=f"z1_{c}")
                nc.tensor.matmul(
                    z1,
                    lhsT=w1_sb[:, 128 * c : 128 * (c + 1)].bitcast(F32R),
                    rhs=xaT_sb.bitcast(F32R),
                    start=True,
                    stop=True,
                )
                nc.scalar.activation(
                    h1_sb[:, BS * c : BS * (c + 1)].bitcast(F32R),
                    z1,
                    AF.Tanh,
                    bias=b12_sb[:, c : c + 1],
                )

            # ---- layer 2: h2^T = tanh(W2^T @ h1^T + b2) -----------------
            h2_sb = acts.tile([128, NCH * BS], F32, tag="h2_sb")
            for c in range(NCH):
                z2 = ps_z.tile([128, BS], F32, tag="z", name=f"z2_{c}")
                for k in range(NCH):
                    nc.tensor.matmul(
                        z2,
                        lhsT=w2_sb[:, k, 128 * c : 128 * (c + 1)].bitcast(F32R),
                        rhs=h1_sb[:, BS * k : BS * (k + 1)].bitcast(F32R),
                        start=(k == 0),
                        stop=(k == NCH - 1),
                    )
                nc.scalar.activation(
                    h2_sb[:, BS * c : BS * (c + 1)].bitcast(F32R),
                    z2,
                    AF.Tanh,
                    bias=b12_sb[:, NCH + c : NCH + c + 1],
                )

            # ---- d = 1 - h^2 (wide ops; squares split ACT/DVE) ----------
            HB = NCH * BS // 2
            hsq1 = acts.tile([128, NCH * BS], F32, tag="hsq1")
            d1_sb = acts.tile([128, NCH * BS], F32, tag="d1_sb")
            for hh in range(2):
                nc.vector.tensor_mul(
                    hsq1[:, HB * hh : HB * (hh + 1)],
                    h1_sb[:, HB * hh : HB * (hh + 1)],
                    h1_sb[:, HB * hh : HB * (hh + 1)],
                )
                nc.vector.tensor_scalar(
                    out=d1_sb[:, HB * hh : HB * (hh + 1)].bitcast(F32R),
                    in0=hsq1[:, HB * hh : HB * (hh + 1)],
                    scalar1=-1.0, scalar2=1.0, op0=OP.mult, op1=OP.add,
                )
            hsq2 = acts.tile([128, NCH * BS], F32, tag="hsq2")
            d2_sb = acts.tile([128, NCH * BS], F32, tag="d2_sb")
            for hh in range(2):
                nc.vector.tensor_mul(
                    hsq2[:, HB * hh : HB * (hh + 1)],
                    h2_sb[:, HB * hh : HB * (hh + 1)],
                    h2_sb[:, HB * hh : HB * (hh + 1)],
                )
                nc.vector.tensor_scalar(
                    out=d2_sb[:, HB * hh : HB * (hh + 1)],
                    in0=hsq2[:, HB * hh : HB * (hh + 1)],
                    scalar1=-1.0, scalar2=1.0, op0=OP.mult, op1=OP.add,
                )

            # ---- layer 3: dx^T = (W3^T @ h2^T + b3) / 2 -----------------
            dx_ps = ps_o.tile([D, BS], F32, tag="dx")
            for k in range(NCH):
                nc.tensor.matmul(
                    dx_ps,
                    lhsT=w3_sb[:, k, :].bitcast(F32R),
                    rhs=h2_sb[:, BS * k : BS * (k + 1)].bitcast(F32R),
                    start=(k == 0),
                    stop=(k == NCH - 1),
                )
            dx_out = acts.tile([D, BS], F32, tag="dx_out")
            nc.vector.tensor_scalar(
                out=dx_out, in0=dx_ps, scalar1=b3o_sb[0:D, 0:1], scalar2=0.5,
                op0=OP.add, op1=OP.mult,
            )
            nc.sync.dma_start(out=out_dx[:, :], in_=dx_out)

            # ---- divergence: div = (d1^T K d2) / 2 ----------------------
            # A^T[j,b] = sum_m K[m,j] d1^T[m,b]  (K is lhsT-native)
            p_sb = acts.tile([128, NCH * BS], F32, tag="p_sb")
            for c in range(NCH):
                a_ps = ps_a.tile([128, BS], F32, tag="a", name=f"a_{c}")
                for k in range(NCH):
                    nc.tensor.matmul(
                        a_ps,
                        lhsT=km_sb[:, k, 128 * c : 128 * (c + 1)].bitcast(F32R),
                        rhs=d1_sb[:, BS * k : BS * (k + 1)].bitcast(F32R),
                        start=(k == 0),
                        stop=(k == NCH - 1),
                    )
                nc.vector.tensor_mul(
                    p_sb[:, BS * c : BS * (c + 1)].bitcast(F32R),
                    a_ps,
                    d2_sb[:, BS * c : BS * (c + 1)],
                )

            # partition-dim reduction via ones-vector matmul
            dv_ps = ps_o.tile([1, BS], F32, tag="dv")
            for c in range(NCH):
                nc.tensor.matmul(
                    dv_ps,
                    lhsT=b3o_sb[:, 1:2].bitcast(F32R),
                    rhs=p_sb[:, BS * c : BS * (c + 1)].bitcast(F32R),
                    start=(c == 0),
                    stop=(c == NCH - 1),
                )
            dv_out = acts.tile([1, BS], F32, tag="dv_out")
            nc.vector.tensor_scalar(
                out=dv_out, in0=dv_ps, scalar1=0.5, scalar2=None, op0=OP.mult,
            )
            nc.gpsimd.dma_start(out=out_dv[:, :], in_=dv_out)

    return nc


_NC = None


def _get_program():
    global _NC
    if _NC is None:
        _NC = _build_program()
    return _NC


def _host_prep(t, x, W1, b1, W2, b2, W3, b3):
    """Shard + lay out inputs for the device program (host does layout only,
    plus the launch-invariant weight fold K)."""
    t = np.asarray(t, np.float32)
    x = np.asarray(x, np.float32)
    W1 = np.asarray(W1, np.float32)
    W2 = np.asarray(W2, np.float32)
    W3 = np.asarray(W3, np.float32)
    b1 = np.asarray(b1, np.float32)
    b2 = np.asarray(b2, np.float32)
    b3 = np.asarray(b3, np.float32)

    # transposed, time-augmented input: rows 0..63 = x^T, row 64 = t
    xaT = np.empty((D + 1, B), np.float32)
    xaT[:D] = x[:, :D].T
    xaT[D] = t[0]

    PK1W, PK2W = 768, 266
    pk1 = np.zeros((D + 1, PK1W), np.float32)
    pk1[:, BS : BS + H] = W1

    pk2 = np.zeros((128, PK2W), np.float32)
    pk2[:, 0 : NCH * D] = W3.reshape(NCH, 128, D).transpose(1, 0, 2).reshape(
        128, NCH * D
    )
    pk2[:, NCH * D : NCH * D + NCH] = b1.reshape(NCH, 128).T
    pk2[:, NCH * D + NCH : NCH * D + 2 * NCH] = b2.reshape(NCH, 128).T
    pk2[:D, PK2W - 2] = b3
    pk2[:, PK2W - 1] = 1.0

    # weight fold: K[m,j] = W2[m,j] * (W1[:D]^T @ W3^T)[m,j]
    kmh = (W2 * (W1[:D].T @ W3.T)).astype(np.float32)

    w2c = np.ascontiguousarray(W2)
    kmc = np.ascontiguousarray(kmh)
    in_maps = []
    for c in range(NCORES):
        p = pk1.copy()
        p[:, 0:BS] = xaT[:, BS * c : BS * (c + 1)]
        in_maps.append({"pk1": p, "pk2": pk2, "w2": w2c, "km": kmc})
    return in_maps


def kernel(t, x, W1, b1, W2, b2, W3, b3):
    nc = _get_program()
    in_maps = _host_prep(t, x, W1, b1, W2, b2, W3, b3)
    res = run_bass_kernel_spmd(nc, in_maps, core_ids=list(range(NCORES)))
    out = np.empty((B, D + 1), np.float32)
    for c in range(NCORES):
        sl = slice(BS * c, BS * (c + 1))
        out[sl, :D] = res.results[c]["out_dx"].T
        out[sl, D] = res.results[c]["out_dv"][0]
    return out
